# revision 1
# baseline (speedup 1.0000x reference)
"""CliffordLinear (Cl(3,0)) Trainium2 kernel.

Math: Cl(3,0) is isomorphic to the algebra of 2x2 complex matrices via the
Pauli-matrix representation phi(x) = sum_a x_a * (s1^b0 s2^b1 s3^b2).  The
reference computes out[b,o] = sum_i W[o,i] * X[b,i] (Clifford product per
channel pair), which maps to OutM[b,o] = sum_i phi(W[o,i]) @ phi(X[b,i]) --
a 2x2 complex matrix contraction.  Splitting by output column c and
expanding complex arithmetic into real matmuls gives, per c in {0,1}:

    OutRe_c[b,(o,r)] = XRe_c @ R - XIm_c @ I
    OutIm_c[b,(o,r)] = XRe_c @ I + XIm_c @ R

with R/I = Re/Im of phi(W)[r,m] as [(i,m) x (o,r)] 512x512 matrices.  That
is 17.2G real MACs total vs 34.4G for the naive blade expansion (2x fewer).
The blade <-> Pauli basis changes are 8-point +- butterflies: the input side
is folded into host-side shard prep; the output side runs on the DVE while
evicting PSUM.  Matmuls run in float32r (TF32-like, ~1.5e-4 rel err, full
PE rate; plain fp32 is 4x slower).

Sharding: data-parallel over batch (1024 rows/core); weights replicated.
Per-core HBM traffic: 8.4 MB x + 2.1 MB w in, 8.4 MB out.
"""

import sys

sys.path.insert(0, "/opt/trn_rl_repo")

import numpy as np

import concourse.bass as bass  # noqa: F401  (registers lowerings)
import concourse.mybir as mybir
import concourse.tile as tile
from concourse import bacc
from concourse.bass_utils import run_bass_kernel_spmd

N_CORES = 8
B, CIN, COUT, NB = 8192, 256, 256, 8
BS = B // N_CORES          # 1024 batch rows per core
K = CIN * 4                # 1024 contraction rows (both halves)
HK = K // 2                # 512: rows per Re/Im half
OUTW = COUT * NB           # 2048 output width (o major, blade minor)
KT = K // 128              # 8 k-tiles of the x operand
BT = BS // 128             # 8 b-tiles

_cached = {}


def _build_nc():
    fr = mybir.dt.float32r
    f32 = mybir.dt.float32
    nc = bacc.Bacc("TRN2", target_bir_lowering=False, debug=False,
                   num_devices=N_CORES)
    # x'[c] layout: [bt, p, k, b] so each per-partition row is 4 KiB contiguous
    xt0 = nc.dram_tensor("xt0", [BT, 128, KT * 128], f32, kind="ExternalInput")
    xt1 = nc.dram_tensor("xt1", [BT, 128, KT * 128], f32, kind="ExternalInput")
    # weight planes R|I stacked: [2, 512, 512] = [Re/Im, (i,m), (o,r)]
    wri = nc.dram_tensor("wri", [2, HK, HK], f32, kind="ExternalInput")
    out = nc.dram_tensor("out", [BS, OUTW], f32, kind="ExternalOutput")

    with tile.TileContext(nc) as tc:
        with tc.tile_pool(name="wpool", bufs=1) as wpool, \
             tc.tile_pool(name="xpool", bufs=4) as xpool, \
             tc.tile_pool(name="opool", bufs=3) as opool, \
             tc.tile_pool(name="pspool", bufs=2, space="PSUM") as pspool:
            # PE warmup: ramp the clock gate during the initial DMA wait so
            # real matmuls start at full speed.  Zeros in, result unused.
            warm_in = wpool.tile([128, 640], mybir.dt.bfloat16, tag="warm_in")
            nc.vector.memset(warm_in[:], 0.0)
            warm_ps = pspool.tile([128, 512], f32, tag="ps0")
            for _ in range(8):
                nc.tensor.matmul(warm_ps[:], warm_in[:, :128], warm_in[:, 128:640],
                                 start=True, stop=True)

            # Startup interleave: bt0's x0 arrives in two 256 KiB chunks
            # around the weight-plane DMAs, so the first matmuls begin
            # after ~1.5 us of DMA instead of after the full 3 MB preload.
            x1_pre = xpool.tile([128, KT * 128], fr, tag="x1")
            x0_chunks = []
            for h in range(2):
                x0ph = xpool.tile([128, 512], fr, tag=f"x0p{h}", bufs=1)
                x0_chunks.append(x0ph)
            nc.sync.dma_start(x0_chunks[0][:], xt0[0][:, 0:512].bitcast(fr))
            r_t, i_t, ni_t = [], [], []
            for k in range(4):
                ik = wpool.tile([128, HK], fr, tag=f"i{k}")
                nc.sync.dma_start(ik[:], wri[1, k * 128:(k + 1) * 128, :].bitcast(fr))
                rk = wpool.tile([128, HK], fr, tag=f"r{k}")
                nc.sync.dma_start(rk[:], wri[0, k * 128:(k + 1) * 128, :].bitcast(fr))
                nik = wpool.tile([128, HK], fr, tag=f"ni{k}")
                nc.scalar.mul(nik[:], ik[:].bitcast(f32), -1.0)
                r_t.append(rk); i_t.append(ik); ni_t.append(nik)
            # x0's second chunk is first needed at k=4, after all w-planes
            nc.sync.dma_start(x0_chunks[1][:], xt0[0][:, 512:1024].bitcast(fr))
            nc.sync.dma_start(x1_pre[:], xt1[0].bitcast(fr))
            # rhs per (half, k): Re half: [R0..R3, -I0..-I3]; Im: [I0..I3, R0..R3]
            rhs_re = r_t + ni_t
            rhs_im = i_t + r_t

            for bt in range(BT):
                if bt == 0:
                    x0_lhs = [x0_chunks[k // 4][:, (k % 4) * 128:(k % 4 + 1) * 128]
                              for k in range(KT)]
                    x1_s = x1_pre
                else:
                    x0_s = xpool.tile([128, KT * 128], fr, tag="x0")
                    x1_s = xpool.tile([128, KT * 128], fr, tag="x1")
                    nc.sync.dma_start(x0_s[:], xt0[bt].bitcast(fr))
                    nc.sync.dma_start(x1_s[:], xt1[bt].bitcast(fr))
                    x0_lhs = [x0_s[:, k * 128:(k + 1) * 128] for k in range(KT)]
                x1_lhs = [x1_s[:, k * 128:(k + 1) * 128] for k in range(KT)]
                ps0 = pspool.tile([128, K], f32, tag="ps0")
                ps1 = pspool.tile([128, K], f32, tag="ps1")
                last = bt == BT - 1
                if not last:
                    for xlhs, ps in ((x0_lhs, ps0), (x1_lhs, ps1)):
                        for k in range(KT):
                            # Im first: its rhs never depends on the ScalarE
                            # negation, so a late nI_k can't stall it in the
                            # PE queue.
                            nc.tensor.matmul(ps[:, HK:K], xlhs[k], rhs_im[k][:],
                                             start=(k == 0), stop=(k == KT - 1))
                            nc.tensor.matmul(ps[:, 0:HK], xlhs[k], rhs_re[k][:],
                                             start=(k == 0), stop=(k == KT - 1))
                else:
                    # c1 first (so its eviction overlaps c0), and c0 split in
                    # two column chunks with separate PSUM tiles so chunk A's
                    # butterfly+store overlap chunk B's matmuls.
                    for k in range(KT):
                        nc.tensor.matmul(ps1[:, HK:K], x1_lhs[k], rhs_im[k][:],
                                         start=(k == 0), stop=(k == KT - 1))
                        nc.tensor.matmul(ps1[:, 0:HK], x1_lhs[k], rhs_re[k][:],
                                         start=(k == 0), stop=(k == KT - 1))
                    ps0a = ps0  # reuse the already-allocated ps0 slot: chunk A
                    ps0b = pspool.tile([128, K], f32, tag="ps1")
                    # Re chunk in bank 0, Im chunk in bank 1 (interleaved
                    # accumulation groups must not share a PSUM bank)
                    for cs, pst in ((0, ps0a), (1, ps0b)):
                        for k in range(KT):
                            nc.tensor.matmul(
                                pst[:, 0:256], x0_lhs[k],
                                rhs_re[k][:, cs * 256:(cs + 1) * 256],
                                start=(k == 0), stop=(k == KT - 1))
                            nc.tensor.matmul(
                                pst[:, HK:HK + 256], x0_lhs[k],
                                rhs_im[k][:, cs * 256:(cs + 1) * 256],
                                start=(k == 0), stop=(k == KT - 1))
                stage = opool.tile([128, OUTW], f32, tag="stage")
                # DVE reads only one PSUM operand: evict ps1 via ScalarE
                s1 = opool.tile([128, K], f32, tag="s1")
                nc.scalar.copy(s1[:], ps1[:])
                # inverse Pauli butterfly into blade-minor layout.
                # ps cols: [Re(o,r) | Im(o,r)], (o,r) packed o*2+r.
                # A=P00 (ps0,r0)  C=P10 (ps0,r1)  B=P01 (ps1,r0)  D=P11 (ps1,r1)
                # 4 dual-blade ops via 2-dim free APs (j picks Re/Im half):
                #   add (x0,x7): out 8o+7j      = ps0[512j+2o]   + s1[512j+2o+1]
                #   sub (x4,x3): out 8o+4-j     = ps0[512j+2o]   - s1[512j+2o+1]
                #   add (x1,x6): out 8o+1+5j    = ps0[512j+2o+1] + s1[512j+2o]
                #   sub (x5,x2): out 8o+5-3j    = ps0[512j+2o+1] - s1[512j+2o]
                def _ap3(base, off, jstep, ostep, ocnt):
                    a = base.copy()
                    part = a.ap.to_list()[0]
                    v = a.ap
                    v.clear()
                    v.extend([tuple(part), (jstep, 2), (ostep, ocnt)])
                    a.offset = a.offset + off
                    return a
                add, sub = nc.vector.tensor_add, nc.vector.tensor_sub
                if not last:
                    chunks = [(ps0, 0, HK, 0, 256, nc.sync)]
                else:
                    chunks = [(ps0a, 0, HK, 0, 128, nc.sync),
                              (ps0b, 0, HK, 256, 128, nc.scalar)]
                for pst, po, pjstep, so1, ocnt, dma_eng in chunks:
                    so = so1 * 4              # stage column offset of chunk
                    add(_ap3(stage[:], so + 0, 7, 8, ocnt),
                        _ap3(pst[:], po + 0, pjstep, 2, ocnt),
                        _ap3(s1[:], so1 + 1, HK, 2, ocnt))
                    sub(_ap3(stage[:], so + 4, -1, 8, ocnt),
                        _ap3(pst[:], po + 0, pjstep, 2, ocnt),
                        _ap3(s1[:], so1 + 1, HK, 2, ocnt))
                    add(_ap3(stage[:], so + 1, 5, 8, ocnt),
                        _ap3(pst[:], po + 1, pjstep, 2, ocnt),
                        _ap3(s1[:], so1 + 0, HK, 2, ocnt))
                    sub(_ap3(stage[:], so + 5, -3, 8, ocnt),
                        _ap3(pst[:], po + 1, pjstep, 2, ocnt),
                        _ap3(s1[:], so1 + 0, HK, 2, ocnt))
                    if last and so1 == 256:
                        # tail-critical store: two queues in parallel
                        half = ocnt * 4
                        nc.scalar.dma_start(
                            out[bt * 128:(bt + 1) * 128, so:so + half],
                            stage[:, so:so + half])
                        nc.sync.dma_start(
                            out[bt * 128:(bt + 1) * 128, so + half:so + ocnt * 8],
                            stage[:, so + half:so + ocnt * 8])
                    else:
                        dma_eng.dma_start(
                            out[bt * 128:(bt + 1) * 128, so:so + ocnt * 8],
                            stage[:, so:so + ocnt * 8])
    nc.finalize()
    return nc


def _pauli_parts(v):
    """v[..., 8] -> c0, c1 of shape [..., 2(m), 2(reim)]: the c-th column
    (Re, Im) of phi(v) rows m.  phi entries: A=P00=(v0+v4)+i(v3+v7),
    B=P01=(v1-v5)+i(v6-v2), C=P10=(v1+v5)+i(v6+v2), D=P11=(v0-v4)+i(v7-v3)."""
    c0 = np.empty(v.shape[:-1] + (2, 2), dtype=v.dtype)
    c1 = np.empty_like(c0)
    v0, v1, v2, v3, v4, v5, v6, v7 = (v[..., a] for a in range(8))
    c0[..., 0, 0] = v0 + v4   # Re A
    c0[..., 0, 1] = v3 + v7   # Im A
    c0[..., 1, 0] = v1 + v5   # Re C
    c0[..., 1, 1] = v6 + v2   # Im C
    c1[..., 0, 0] = v1 - v5   # Re B
    c1[..., 0, 1] = v6 - v2   # Im B
    c1[..., 1, 0] = v0 - v4   # Re D
    c1[..., 1, 1] = v7 - v3   # Im D
    return c0, c1


def _prep_w(weight):
    """weight [COUT, CIN, 8] -> [2, 512, 512] stacked R|I planes of
    phi(W)[r,m] indexed [(i,m), (o,r)], with the 0.5 inverse factor folded."""
    w = weight.astype(np.float32)
    # _pauli_parts returns matrix COLUMNS: cw_m[o,i,r,:] = (Re, Im) of
    # phi(W[o,i])[r, m].
    cw0, cw1 = _pauli_parts(w)
    R = np.empty((CIN, 2, COUT, 2), np.float32)   # [(i,m),(o,r)]
    I = np.empty_like(R)
    for m, cm in ((0, cw0), (1, cw1)):
        for r in range(2):
            R[:, m, :, r] = 0.5 * cm[:, :, r, 0].T
            I[:, m, :, r] = 0.5 * cm[:, :, r, 1].T
    return np.ascontiguousarray(
        np.stack([R.reshape(HK, HK), I.reshape(HK, HK)], axis=0))


def _prep_x(x):
    """x [B, CIN, 8] -> per-core xt arrays [N_CORES][BT, 128, KT*128] for
    c=0 and c=1, in the [bt, p, k, b] DMA-friendly layout.  Contraction row
    kappa = half*512 + i*2 + m  (half = 0:Re, 1:Im)."""
    xf = x.astype(np.float32)
    c0, c1 = _pauli_parts(xf)          # [B, CIN, m, reim]
    outs = []
    for arr in (c0, c1):
        # kappa-major array [K, B]: a = i*2+m ; kappa = ri*512 + a
        kb = arr.transpose(3, 1, 2, 0).reshape(K, B)   # [ri, i, m, b] -> [K, B]
        # device layout [core, bt, p, k, b]; kappa = k*128 + p
        a = kb.reshape(KT, 128, N_CORES, BT, 128)       # [k, p, core, bt, b]
        a = a.transpose(2, 3, 1, 0, 4)                  # [core, bt, p, k, b]
        outs.append(np.ascontiguousarray(
            a.reshape(N_CORES, BT, 128, KT * 128)))
    return outs


def kernel(x, weight, bias, cayley):
    assert x.shape == (B, CIN, NB) and weight.shape == (COUT, CIN, NB)
    if "nc" not in _cached:
        _cached["nc"] = _build_nc()
    nc = _cached["nc"]

    xt0, xt1 = _prep_x(np.asarray(x))
    wri = _prep_w(np.asarray(weight))
    in_maps = [{"xt0": xt0[c], "xt1": xt1[c], "wri": wri} for c in range(N_CORES)]
    res = run_bass_kernel_spmd(nc, in_maps, core_ids=list(range(N_CORES)))
    out = np.concatenate([res.results[c]["out"] for c in range(N_CORES)], axis=0)
    out = out.reshape(B, COUT, NB) + np.asarray(bias, np.float32)[None]
    return out.astype(np.float32)



# revision 2
# speedup vs baseline: 1.2154x; 1.2154x over previous
"""CliffordLinear (Cl(3,0)) Trainium2 kernel — fp8 DoubleRow edition.

Math: Cl(3,0) ~= 2x2 complex matrices via Pauli rep phi.  The reference's
per-channel Clifford contraction maps to OutM[b,o] = sum_i phi(W[o,i]) @
phi(X[b,i]).  Per output column c in {0,1} and with R/I = Re/Im of phi(W)
as [(i,m) x (o,r)] 512x512 matrices:

    OutRe_c = XRe_c @ R - XIm_c @ I        (psum cols [0,512))
    OutIm_c = XRe_c @ I + XIm_c @ R        (psum cols [512,1024))

Precision/performance trick: all matmul operands are fp8e4 (e4m3) hi+lo
pairs prepared on the host: v ~= hi + lo with hi = e4m3(v), lo = e4m3(v-hi)
(~13 effective mantissa bits, ~1e-3 rel err).  The product expands to three
fp8 passes (hi*hi + lo*hi + hi*lo; the lo*lo term is ~1e-3 relative and
dropped).  fp8 pairs run the PE in DoubleRow perf mode: each matmul
contracts 256 rows (2 k-tiles) at 0.5 cycles/row -- 2x the bf16/f32r
column rate, so 3 passes cost 0.75x the single-pass f32r time.  Inputs and
outputs move over HBM at half width too (fp8 pairs in, bf16 out).

Sharding: data-parallel over batch (1024 rows/core); weights replicated.
Per-core HBM traffic: 4.2 MB x + 1.5 MB w in, 4.2 MB out.
"""

import sys

sys.path.insert(0, "/opt/trn_rl_repo")

import numpy as np
import ml_dtypes

import concourse.bass as bass  # noqa: F401  (registers lowerings)
import concourse.mybir as mybir
import concourse.tile as tile
from concourse import bacc
from concourse.bass_utils import run_bass_kernel_spmd

N_CORES = 8
B, CIN, COUT, NB = 8192, 256, 256, 8
BS = B // N_CORES          # 1024 batch rows per core
K = CIN * 4                # 1024 contraction rows (re|im halves stacked)
HK = K // 2                # 512 rows per Re/Im half
OUTW = COUT * NB           # 2048 output width (o major, blade minor)
KT = K // 128              # 8 k-tiles
BT = BS // 128             # 8 b-tiles
E4 = ml_dtypes.float8_e4m3
SX = 16.0                  # x scale before e4m3 quantization
SW = 1024.0                # w scale before e4m3 quantization

_cached = {}


def _build_nc():
    f32 = mybir.dt.float32
    bf16 = mybir.dt.bfloat16
    fp8 = mybir.dt.float8e4
    DR = mybir.MatmulPerfMode.DoubleRow
    nc = bacc.Bacc("TRN2", target_bir_lowering=False, debug=False,
                   num_devices=N_CORES)
    # x'[c]: [bt, p, hl, kt, b] flattened to [bt, 128, 2048] fp8.
    # kappa = kt*128 + p; hl = 0 (hi plane) / 1 (lo plane).
    xt0 = nc.dram_tensor("xt0", [BT, 128, 2048], fp8, kind="ExternalInput")
    xt1 = nc.dram_tensor("xt1", [BT, 128, 2048], fp8, kind="ExternalInput")
    # w kpair blocks [R01, R23, -I01, -I23, I01, I23]; each block [p, t, c]
    # with row kappa_local = t*128 + p, col (o,r).  [hl, 128, 6*2*512].
    w8 = nc.dram_tensor("w8", [2, 128, 6144], fp8, kind="ExternalInput")
    out = nc.dram_tensor("out", [BS, OUTW], bf16, kind="ExternalOutput")

    # w kpair-block index per (output half, kappa-pair j)
    KPIDX = {0: [0, 1, 2, 3],   # re out: [R ; -I]
             1: [4, 5, 0, 1]}   # im out: [I ; R]

    with tile.TileContext(nc) as tc:
        with tc.tile_pool(name="wpool", bufs=1) as wpool, \
             tc.tile_pool(name="xpool", bufs=3) as xpool, \
             tc.tile_pool(name="opool", bufs=3) as opool, \
             tc.tile_pool(name="pspool", bufs=2, space="PSUM") as pspool:
            # PE warmup: ramp the clock gate during the initial DMA wait.
            warm_in = wpool.tile([128, 640], bf16, tag="warm_in")
            nc.vector.memset(warm_in[:], 0.0)
            warm_ps = pspool.tile([128, 512], f32, tag="ps0")
            for _ in range(8):
                nc.tensor.matmul(warm_ps[:], warm_in[:, :128], warm_in[:, 128:640],
                                 start=True, stop=True)

            # Startup: x0's hi plane first (pass 1), then w_hi, then the
            # rest, so the first matmuls begin ~2.5us in.
            wh = wpool.tile([128, 6, 2, 512], fp8, tag="wh")
            wl = wpool.tile([128, 6, 2, 512], fp8, tag="wl")
            x0_pre = xpool.tile([128, 2, 8, 128], fp8, tag="x0")
            x1_pre = xpool.tile([128, 2, 8, 128], fp8, tag="x1")
            nc.sync.dma_start(x0_pre[:, 0], xt0[0][:, 0:1024])
            nc.sync.dma_start(wh[:], w8[0])
            nc.sync.dma_start(x0_pre[:, 1], xt0[0][:, 1024:2048])
            nc.sync.dma_start(wl[:], w8[1])
            nc.sync.dma_start(x1_pre[:], xt1[0])

            for bt in range(BT):
                if bt == 0:
                    x0_s, x1_s = x0_pre, x1_pre
                else:
                    x0_s = xpool.tile([128, 2, 8, 128], fp8, tag="x0")
                    x1_s = xpool.tile([128, 2, 8, 128], fp8, tag="x1")
                    nc.sync.dma_start(x0_s[:], xt0[bt])
                    nc.sync.dma_start(x1_s[:], xt1[bt])
                ps0 = pspool.tile([128, K], f32, tag="ps0")
                ps1 = pspool.tile([128, K], f32, tag="ps1")
                for xs, ps in ((x0_s, ps0), (x1_s, ps1)):
                    for ch in (0, 2, 1, 3):   # alternate psum banks
                        half = 0 if ch < 2 else 1
                        col0 = 256 * ch
                        wc0 = 256 * (ch % 2)
                        kpidx = KPIDX[half]
                        for pi, (hlx, wt) in enumerate(
                                ((0, wh), (1, wh), (0, wl))):
                            for j in range(4):
                                nc.tensor.matmul(
                                    ps[:, col0:col0 + 256],
                                    xs[:, hlx, 2 * j:2 * j + 2, :],
                                    wt[:, kpidx[j], :, wc0:wc0 + 256],
                                    start=(pi == 0 and j == 0),
                                    stop=(pi == 2 and j == 3),
                                    perf_mode=DR)
                stage = opool.tile([128, OUTW], bf16, tag="stage")
                # DVE reads only one PSUM operand: evict ps1 via ScalarE
                s1 = opool.tile([128, K], f32, tag="s1")
                nc.scalar.copy(s1[:], ps1[:])
                # inverse Pauli butterfly into blade-minor bf16 layout.
                # psum cols: [Re(o,r) | Im(o,r)], (o,r) packed o*2+r.
                #   add (x0,x7): out 8o+7j   = ps0[512j+2o]   + s1[512j+2o+1]
                #   sub (x4,x3): out 8o+4-j  = ps0[512j+2o]   - s1[512j+2o+1]
                #   add (x1,x6): out 8o+1+5j = ps0[512j+2o+1] + s1[512j+2o]
                #   sub (x5,x2): out 8o+5-3j = ps0[512j+2o+1] - s1[512j+2o]
                def _ap3(base, off, jstep, ostep, ocnt):
                    a = base.copy()
                    part = a.ap.to_list()[0]
                    v = a.ap
                    v.clear()
                    v.extend([tuple(part), (jstep, 2), (ostep, ocnt)])
                    a.offset = a.offset + off
                    return a
                add, sub = nc.vector.tensor_add, nc.vector.tensor_sub
                add(_ap3(stage[:], 0, 7, 8, 256),
                    _ap3(ps0[:], 0, HK, 2, 256),
                    _ap3(s1[:], 1, HK, 2, 256))
                sub(_ap3(stage[:], 4, -1, 8, 256),
                    _ap3(ps0[:], 0, HK, 2, 256),
                    _ap3(s1[:], 1, HK, 2, 256))
                add(_ap3(stage[:], 1, 5, 8, 256),
                    _ap3(ps0[:], 1, HK, 2, 256),
                    _ap3(s1[:], 0, HK, 2, 256))
                sub(_ap3(stage[:], 5, -3, 8, 256),
                    _ap3(ps0[:], 1, HK, 2, 256),
                    _ap3(s1[:], 0, HK, 2, 256))
                nc.sync.dma_start(out[bt * 128:(bt + 1) * 128, :], stage[:])
    nc.finalize()
    return nc


def _pauli_parts(v):
    """v[..., 8] -> c0, c1 of shape [..., 2(m), 2(reim)]: the c-th column
    (Re, Im) of phi(v) rows m."""
    c0 = np.empty(v.shape[:-1] + (2, 2), dtype=v.dtype)
    c1 = np.empty_like(c0)
    v0, v1, v2, v3, v4, v5, v6, v7 = (v[..., a] for a in range(8))
    c0[..., 0, 0] = v0 + v4   # Re A
    c0[..., 0, 1] = v3 + v7   # Im A
    c0[..., 1, 0] = v1 + v5   # Re C
    c0[..., 1, 1] = v6 + v2   # Im C
    c1[..., 0, 0] = v1 - v5   # Re B
    c1[..., 0, 1] = v6 - v2   # Im B
    c1[..., 1, 0] = v0 - v4   # Re D
    c1[..., 1, 1] = v7 - v3   # Im D
    return c0, c1


def _hi_lo(v):
    """f32 array -> (hi, lo) e4m3 planes with hi + lo ~= v."""
    hi = v.astype(E4)
    lo = (v - hi.astype(np.float32)).astype(E4)
    return hi, lo


def _prep_w(weight):
    """weight [COUT, CIN, 8] -> [2, 128, 6144] fp8 kpair blocks
    [R01, R23, -I01, -I23, I01, I23], each [128 p, 2 t, 512 c]."""
    w = weight.astype(np.float32)
    cw0, cw1 = _pauli_parts(w)
    R = np.empty((CIN, 2, COUT, 2), np.float32)   # [(i,m),(o,r)]
    I = np.empty_like(R)
    for m, cm in ((0, cw0), (1, cw1)):
        for r in range(2):
            R[:, m, :, r] = 0.5 * cm[:, :, r, 0].T
            I[:, m, :, r] = 0.5 * cm[:, :, r, 1].T
    R = R.reshape(HK, HK) * SW
    I = I.reshape(HK, HK) * SW
    out = np.empty((2, 128, 6144), dtype=E4)
    for hl, quant in ((0, 0), (1, 1)):
        blocks = []
        for M in (R, -I, I):
            h, l = _hi_lo(M)
            P = (h if quant == 0 else l).astype(np.float32)
            for j in (0, 1):
                blk = P[256 * j:256 * j + 256].reshape(2, 128, HK)
                blocks.append(blk.transpose(1, 0, 2))   # [128, 2, 512]
        # order: R01, R23, nI01, nI23, I01, I23
        order = [0, 1, 2, 3, 4, 5]
        arr = np.stack([blocks[i] for i in order], axis=1)  # [128, 6, 2, 512]
        out[hl] = arr.reshape(128, 6144).astype(E4)
    return out


def _prep_x(x):
    """x [B, CIN, 8] -> two per-core arrays [N_CORES, BT, 128, 2048] fp8
    (c = 0, 1) in [bt, p, hl, kt, b] layout; kappa = kt*128 + p."""
    xf = x.astype(np.float32)
    c0, c1 = _pauli_parts(xf)          # [B, CIN, m, reim]
    outs = []
    for arr in (c0, c1):
        kb = arr.transpose(3, 1, 2, 0).reshape(K, B) * SX   # [K, B]
        a = kb.reshape(KT, 128, N_CORES, BT, 128)   # [kt, p, core, bt, b]
        a = np.ascontiguousarray(a.transpose(2, 3, 1, 0, 4))  # [core,bt,p,kt,b]
        hi, lo = _hi_lo(a)
        packed = np.stack([hi, lo], axis=3)  # [core, bt, p, hl, kt, b]
        outs.append(np.ascontiguousarray(
            packed.reshape(N_CORES, BT, 128, 2048)))
    return outs


def kernel(x, weight, bias, cayley):
    assert x.shape == (B, CIN, NB) and weight.shape == (COUT, CIN, NB)
    if "nc" not in _cached:
        _cached["nc"] = _build_nc()
    nc = _cached["nc"]

    xt0, xt1 = _prep_x(np.asarray(x))
    w8 = _prep_w(np.asarray(weight))
    in_maps = [{"xt0": xt0[c], "xt1": xt1[c], "w8": w8} for c in range(N_CORES)]
    res = run_bass_kernel_spmd(nc, in_maps, core_ids=list(range(N_CORES)))
    out = np.concatenate(
        [res.results[c]["out"].astype(np.float32) for c in range(N_CORES)],
        axis=0)
    out = out.reshape(B, COUT, NB) * (1.0 / (SX * SW))
    out = out + np.asarray(bias, np.float32)[None]
    return out.astype(np.float32)


# revision 4
# speedup vs baseline: 1.2247x; 1.0076x over previous
"""CliffordLinear (Cl(3,0)) Trainium2 kernel — fp8 DoubleRow edition.

Math: Cl(3,0) ~= 2x2 complex matrices via Pauli rep phi.  The reference's
per-channel Clifford contraction maps to OutM[b,o] = sum_i phi(W[o,i]) @
phi(X[b,i]).  Per output column c in {0,1} and with R/I = Re/Im of phi(W)
as [(i,m) x (o,r)] 512x512 matrices:

    OutRe_c = XRe_c @ R - XIm_c @ I        (psum cols [0,512))
    OutIm_c = XRe_c @ I + XIm_c @ R        (psum cols [512,1024))

Precision/performance trick: all matmul operands are fp8e4 (e4m3) hi+lo
pairs prepared on the host: v ~= hi + lo with hi = e4m3(v), lo = e4m3(v-hi)
(~13 effective mantissa bits, ~1e-3 rel err).  The product expands to three
fp8 passes (hi*hi + lo*hi + hi*lo; the dropped lo*lo term is ~1e-3
relative).  fp8 pairs run the PE in DoubleRow perf mode: each matmul
contracts 256 rows (2 k-tiles) at 0.5 cycles/row -- 2x the bf16/f32r
column rate, so 3 passes cost 0.75x a single f32r pass.  Inputs and
outputs move over HBM at half width too (fp8 pairs in, bf16 out).

PSUM eviction: ScalarE copies each psum to SBUF bf16, deinterleaving
(o,r) so the inverse-Pauli butterfly runs on packed bf16 APs (DVE 2x_1p
mode), split across DVE and Pool.  The last b-tile computes c1 before c0
and drains in column halves so the butterfly + store tail mostly overlaps
the final matmuls.

Sharding: data-parallel over batch (1024 rows/core); weights replicated.
Per-core HBM traffic: 4.2 MB x + 2 MB w in, 4.2 MB out.
"""

import sys

sys.path.insert(0, "/opt/trn_rl_repo")

import numpy as np
import ml_dtypes

import concourse.bass as bass  # noqa: F401  (registers lowerings)
import concourse.mybir as mybir
import concourse.tile as tile
from concourse import bacc
from concourse.bass_utils import run_bass_kernel_spmd

N_CORES = 8
B, CIN, COUT, NB = 8192, 256, 256, 8
BS = B // N_CORES          # 1024 batch rows per core
K = CIN * 4                # 1024 contraction rows (re|im halves stacked)
HK = K // 2                # 512 rows per Re/Im half
OUTW = COUT * NB           # 2048 output width
KT = K // 128              # 8 k-tiles
BT = BS // 128             # 8 b-tiles
E4 = ml_dtypes.float8_e4m3
SX = 16.0                  # x scale before e4m3 quantization
SW = 1024.0                # w scale before e4m3 quantization

_cached = {}


def _apn(base, off, dims):
    """AP with explicit free dims [(step, n), ...] at column offset off."""
    a = base.copy()
    part = a.ap.to_list()[0]
    v = a.ap
    v.clear()
    v.extend([tuple(part)] + [tuple(d) for d in dims])
    a.offset = a.offset + off
    return a


def _build_nc():
    f32 = mybir.dt.float32
    bf16 = mybir.dt.bfloat16
    fp8 = mybir.dt.float8e4
    DR = mybir.MatmulPerfMode.DoubleRow
    nc = bacc.Bacc("TRN2", target_bir_lowering=False, debug=False,
                   num_devices=N_CORES)
    # x'[c]: [bt, p, hl, kt, b] flattened to [bt, 128, 2048] fp8.
    # kappa = kt*128 + p; hl = 0 (hi plane) / 1 (lo plane).
    xt0 = nc.dram_tensor("xt0", [BT, 128, 2048], fp8, kind="ExternalInput")
    xt1 = nc.dram_tensor("xt1", [BT, 128, 2048], fp8, kind="ExternalInput")
    # w kpair blocks per (hl, output half): re half [R01, R23, -I01, -I23],
    # im half [I01, I23, R01, R23]; each block [p, t, c], kappa = t*128 + p
    # within pair.  Flattened [2, 2, 128, 4096].
    w8 = nc.dram_tensor("w8", [2, 2, 128, 4096], fp8, kind="ExternalInput")
    # out cols: h*1024 + l*128 + o_loc  (h = o-half, l = blade, o = h*128+o_loc)
    out = nc.dram_tensor("out", [BS, OUTW], bf16, kind="ExternalOutput")

    with tile.TileContext(nc) as tc:
        with tc.tile_pool(name="wpool", bufs=1) as wpool, \
             tc.tile_pool(name="xpool", bufs=3) as xpool, \
             tc.tile_pool(name="opool", bufs=3) as opool, \
             tc.tile_pool(name="pspool", bufs=2, space="PSUM") as pspool:
            # PE warmup: ramp the clock gate during the initial DMA wait.
            warm_in = wpool.tile([128, 640], bf16, tag="warm_in")
            nc.vector.memset(warm_in[:], 0.0)
            warm_ps = pspool.tile([128, 512], f32, tag="ps0")
            for _ in range(8):
                nc.tensor.matmul(warm_ps[:], warm_in[:, :128], warm_in[:, 128:640],
                                 start=True, stop=True)

            # w tiles per (hl, half); DMA order interleaved with bt0's x so
            # every operand lands just before first use.
            wt = [[wpool.tile([128, 4, 2, 512], fp8, tag=f"w{hl}{ha}",
                              name=f"w{hl}{ha}")
                   for ha in range(2)] for hl in range(2)]
            x0_pre = xpool.tile([128, 2, 8, 128], fp8, tag="x0")
            x1_pre = xpool.tile([128, 2, 8, 128], fp8, tag="x1")
            nc.sync.dma_start(x0_pre[:, 0], xt0[0][:, 0:1024])      # x0 hi
            nc.sync.dma_start(wt[0][0][:], w8[0, 0])                # w hi re
            nc.sync.dma_start(x0_pre[:, 1], xt0[0][:, 1024:2048])   # x0 lo
            nc.sync.dma_start(wt[1][0][:], w8[1, 0])                # w lo re
            nc.sync.dma_start(wt[0][1][:], w8[0, 1])                # w hi im
            nc.sync.dma_start(x1_pre[:], xt1[0])
            nc.sync.dma_start(wt[1][1][:], w8[1, 1])                # w lo im

            def chunk_group(xs, ps, ch):
                """One accumulation group: 256 psum cols, 3 fp8 passes."""
                half = 0 if ch < 2 else 1
                col0 = 256 * ch
                wc0 = 256 * (ch % 2)
                for pi, hlw in enumerate((0, 0, 1)):
                    hlx = (0, 1, 0)[pi]
                    w_t = wt[hlw][half]
                    for j in range(4):
                        nc.tensor.matmul(
                            ps[:, col0:col0 + 256],
                            xs[:, hlx, 2 * j:2 * j + 2, :],
                            w_t[:, j, :, wc0:wc0 + 256],
                            start=(pi == 0 and j == 0),
                            stop=(pi == 2 and j == 3),
                            perf_mode=DR)

            def evict(ps, h, tag):
                """psum half h -> SBUF bf16 [j, r, o] packed (cols j*256+r*128+o)."""
                s = opool.tile([128, 512], bf16, tag=tag)
                nc.scalar.copy(
                    _apn(s[:], 0, [(256, 2), (1, 128), (128, 2)]),
                    _apn(ps[:], 256 * h, [(HK, 2), (2, 128), (1, 2)]))
                return s

            def bfly(s0, s1, h, stage):
                """8 blades for o-half h from evicted comps; DVE+Pool split.
                stage cols: 1024h + l*128 + o_loc."""
                base = 1024 * h
                o2 = [(1, 128)]
                nc.vector.tensor_add(                      # x0, x7
                    _apn(stage[:], base + 0 * 128, [(896, 2)] + o2),
                    _apn(s0[:], 0, [(256, 2)] + o2),
                    _apn(s1[:], 128, [(256, 2)] + o2))
                nc.gpsimd.tensor_sub(                      # x4, x3
                    _apn(stage[:], base + 4 * 128, [(-128, 2)] + o2),
                    _apn(s0[:], 0, [(256, 2)] + o2),
                    _apn(s1[:], 128, [(256, 2)] + o2))
                nc.vector.tensor_add(                      # x1, x6
                    _apn(stage[:], base + 1 * 128, [(640, 2)] + o2),
                    _apn(s0[:], 128, [(256, 2)] + o2),
                    _apn(s1[:], 0, [(256, 2)] + o2))
                nc.gpsimd.tensor_sub(                      # x5, x2
                    _apn(stage[:], base + 5 * 128, [(-384, 2)] + o2),
                    _apn(s0[:], 128, [(256, 2)] + o2),
                    _apn(s1[:], 0, [(256, 2)] + o2))

            for bt in range(BT):
                last = bt == BT - 1
                if bt == 0:
                    x0_s, x1_s = x0_pre, x1_pre
                else:
                    x0_s = xpool.tile([128, 2, 8, 128], fp8, tag="x0")
                    x1_s = xpool.tile([128, 2, 8, 128], fp8, tag="x1")
                    nc.sync.dma_start(x0_s[:], xt0[bt])
                    nc.sync.dma_start(x1_s[:], xt1[bt])
                ps0 = pspool.tile([128, K], f32, tag="ps0")
                ps1 = pspool.tile([128, K], f32, tag="ps1")
                stage = opool.tile([128, OUTW], bf16, tag="stage")
                row = out[bt * 128:(bt + 1) * 128, :]
                if bt == 0:
                    # w tiles arrive re-half first: run chunks in 0,1,2,3.
                    for ch in (0, 1, 2, 3):
                        chunk_group(x0_s, ps0, ch)
                    s0a = evict(ps0, 0, "s0a")
                    s0b = evict(ps0, 1, "s0b")
                    for ch in (0, 1, 2, 3):
                        chunk_group(x1_s, ps1, ch)
                    s1a = evict(ps1, 0, "s1a")
                    bfly(s0a, s1a, 0, stage)
                    s1b = evict(ps1, 1, "s1b")
                    bfly(s0b, s1b, 1, stage)
                    nc.sync.dma_start(row, stage[:])
                elif not last:
                    for ch in (0, 2, 1, 3):
                        chunk_group(x0_s, ps0, ch)
                    s0a = evict(ps0, 0, "s0a")
                    s0b = evict(ps0, 1, "s0b")
                    for ch in (0, 2):
                        chunk_group(x1_s, ps1, ch)
                    s1a = evict(ps1, 0, "s1a")
                    bfly(s0a, s1a, 0, stage)
                    for ch in (1, 3):
                        chunk_group(x1_s, ps1, ch)
                    s1b = evict(ps1, 1, "s1b")
                    bfly(s0b, s1b, 1, stage)
                    nc.sync.dma_start(row, stage[:])
                else:
                    # tail: c1 first, drain in column halves so butterfly
                    # and stores overlap the final matmuls.
                    for ch in (0, 2, 1, 3):
                        chunk_group(x1_s, ps1, ch)
                    s1a = evict(ps1, 0, "s1a")
                    s1b = evict(ps1, 1, "s1b")
                    for ch in (0, 2):
                        chunk_group(x0_s, ps0, ch)
                    s0a = evict(ps0, 0, "s0a")
                    bfly(s0a, s1a, 0, stage)
                    nc.sync.dma_start(row[:, 0:1024], stage[:, 0:1024])
                    for ch in (1, 3):
                        chunk_group(x0_s, ps0, ch)
                    s0b = evict(ps0, 1, "s0b")
                    bfly(s0b, s1b, 1, stage)
                    nc.sync.dma_start(row[:, 1024:2048], stage[:, 1024:2048])
    nc.finalize()
    return nc


def _pauli_parts(v):
    """v[..., 8] -> c0, c1 of shape [..., 2(m), 2(reim)]: the c-th column
    (Re, Im) of phi(v) rows m."""
    c0 = np.empty(v.shape[:-1] + (2, 2), dtype=v.dtype)
    c1 = np.empty_like(c0)
    v0, v1, v2, v3, v4, v5, v6, v7 = (v[..., a] for a in range(8))
    c0[..., 0, 0] = v0 + v4   # Re A
    c0[..., 0, 1] = v3 + v7   # Im A
    c0[..., 1, 0] = v1 + v5   # Re C
    c0[..., 1, 1] = v6 + v2   # Im C
    c1[..., 0, 0] = v1 - v5   # Re B
    c1[..., 0, 1] = v6 - v2   # Im B
    c1[..., 1, 0] = v0 - v4   # Re D
    c1[..., 1, 1] = v7 - v3   # Im D
    return c0, c1


def _hi_lo(v):
    """f32 array -> (hi, lo) e4m3 planes with hi + lo ~= v."""
    hi = v.astype(E4)
    lo = (v - hi.astype(np.float32)).astype(E4)
    return hi, lo


def _prep_w(weight):
    """weight [COUT, CIN, 8] -> [2, 2, 128, 4096] fp8 kpair blocks."""
    w = weight.astype(np.float32)
    cw0, cw1 = _pauli_parts(w)
    R = np.empty((CIN, 2, COUT, 2), np.float32)   # [(i,m),(o,r)]
    I = np.empty_like(R)
    for m, cm in ((0, cw0), (1, cw1)):
        for r in range(2):
            R[:, m, :, r] = 0.5 * cm[:, :, r, 0].T
            I[:, m, :, r] = 0.5 * cm[:, :, r, 1].T
    R = R.reshape(HK, HK) * SW
    I = I.reshape(HK, HK) * SW
    Rh, Rl = _hi_lo(R)
    Ih, Il = _hi_lo(-I)         # nI planes
    Jh, Jl = _hi_lo(I)
    out = np.empty((2, 2, 128, 4096), dtype=E4)
    for hl, (Rp, nIp, Ip) in enumerate(((Rh, Ih, Jh), (Rl, Il, Jl))):
        for ha, planes in ((0, (Rp, nIp)), (1, (Ip, Rp))):
            blocks = []
            for M in planes:
                Mf = M.astype(np.float32)
                for j in (0, 1):
                    blk = Mf[256 * j:256 * j + 256].reshape(2, 128, HK)
                    blocks.append(blk.transpose(1, 0, 2))   # [128, 2, 512]
            arr = np.stack(blocks, axis=1)    # [128, 4, 2, 512]
            out[hl, ha] = arr.reshape(128, 4096).astype(E4)
    return out


def _prep_x(x):
    """x [B, CIN, 8] -> two per-core arrays [N_CORES, BT, 128, 2048] fp8
    (c = 0, 1) in [bt, p, hl, kt, b] layout; kappa = kt*128 + p."""
    xf = x.astype(np.float32)
    c0, c1 = _pauli_parts(xf)          # [B, CIN, m, reim]
    outs = []
    for arr in (c0, c1):
        kb = arr.transpose(3, 1, 2, 0).reshape(K, B) * SX   # [K, B]
        a = kb.reshape(KT, 128, N_CORES, BT, 128)   # [kt, p, core, bt, b]
        a = np.ascontiguousarray(a.transpose(2, 3, 1, 0, 4))  # [core,bt,p,kt,b]
        hi, lo = _hi_lo(a)
        packed = np.stack([hi, lo], axis=3)  # [core, bt, p, hl, kt, b]
        outs.append(np.ascontiguousarray(
            packed.reshape(N_CORES, BT, 128, 2048)))
    return outs


def kernel(x, weight, bias, cayley):
    assert x.shape == (B, CIN, NB) and weight.shape == (COUT, CIN, NB)
    if "nc" not in _cached:
        _cached["nc"] = _build_nc()
    nc = _cached["nc"]

    xt0, xt1 = _prep_x(np.asarray(x))
    w8 = _prep_w(np.asarray(weight))
    in_maps = [{"xt0": xt0[c], "xt1": xt1[c], "w8": w8} for c in range(N_CORES)]
    res = run_bass_kernel_spmd(nc, in_maps, core_ids=list(range(N_CORES)))
    out = np.concatenate(
        [res.results[c]["out"].astype(np.float32) for c in range(N_CORES)],
        axis=0)
    # device cols: (h, l, o_loc) -> out[b, o, l] with o = h*128 + o_loc
    out = out.reshape(B, 2, NB, 128).transpose(0, 1, 3, 2).reshape(B, COUT, NB)
    out = out * (1.0 / (SX * SW))
    out = out + np.asarray(bias, np.float32)[None]
    return out.astype(np.float32)


# revision 8
# speedup vs baseline: 1.2563x; 1.0258x over previous
"""CliffordLinear (Cl(3,0)) Trainium2 kernel — fp8 DoubleRow edition.

Math: Cl(3,0) ~= 2x2 complex matrices via Pauli rep phi.  The reference's
per-channel Clifford contraction maps to OutM[b,o] = sum_i phi(W[o,i]) @
phi(X[b,i]).  Per output column c in {0,1} and with R/I = Re/Im of phi(W)
as [(i,m) x (o,r)] 512x512 matrices:

    OutRe_c = XRe_c @ R - XIm_c @ I        (psum cols [0,512))
    OutIm_c = XRe_c @ I + XIm_c @ R        (psum cols [512,1024))

Precision/performance trick: all matmul operands are fp8e4 (e4m3) hi+lo
pairs prepared on the host: v ~= hi + lo with hi = e4m3(v), lo = e4m3(v-hi)
(~13 effective mantissa bits, ~1e-3 rel err).  The product expands to three
fp8 passes (hi*hi + lo*hi + hi*lo; the dropped lo*lo term is ~1e-3
relative).  fp8 pairs run the PE in DoubleRow perf mode: each matmul
contracts 256 rows (2 k-tiles) at 0.5 cycles/row -- 2x the bf16/f32r
column rate, so 3 passes cost 0.75x a single f32r pass.  Inputs and
outputs move over HBM at half width too (fp8 pairs in, bf16 out).

PSUM eviction: ScalarE copies each psum to SBUF bf16, deinterleaving
(o,r) so the inverse-Pauli butterfly runs on packed bf16 APs (DVE 2x_1p
mode), split across DVE and Pool.  The last b-tile computes c1 before c0
and drains in column halves so the butterfly + store tail mostly overlaps
the final matmuls.

Sharding: data-parallel over batch (1024 rows/core); weights replicated.
Per-core HBM traffic: 4.2 MB x + 2 MB w in, 4.2 MB out.
"""

import sys

sys.path.insert(0, "/opt/trn_rl_repo")

import numpy as np
import ml_dtypes

import concourse.bass as bass  # noqa: F401  (registers lowerings)
import concourse.mybir as mybir
import concourse.tile as tile
from concourse import bacc
from concourse.bass_utils import run_bass_kernel_spmd

N_CORES = 8
B, CIN, COUT, NB = 8192, 256, 256, 8
BS = B // N_CORES          # 1024 batch rows per core
K = CIN * 4                # 1024 contraction rows (re|im halves stacked)
HK = K // 2                # 512 rows per Re/Im half
OUTW = COUT * NB           # 2048 output width
KT = K // 128              # 8 k-tiles
BT = BS // 128             # 8 b-tiles
E4 = ml_dtypes.float8_e4m3
SX = 16.0                  # x scale before e4m3 quantization
SW = 1024.0                # w scale before e4m3 quantization

_cached = {}


def _apn(base, off, dims):
    """AP with explicit free dims [(step, n), ...] at column offset off."""
    a = base.copy()
    part = a.ap.to_list()[0]
    v = a.ap
    v.clear()
    v.extend([tuple(part)] + [tuple(d) for d in dims])
    a.offset = a.offset + off
    return a


def _build_nc():
    f32 = mybir.dt.float32
    bf16 = mybir.dt.bfloat16
    fp8 = mybir.dt.float8e4
    DR = mybir.MatmulPerfMode.DoubleRow
    nc = bacc.Bacc("TRN2", target_bir_lowering=False, debug=False,
                   num_devices=N_CORES)
    # x'[c]: [bt, p, hl, kt, b] flattened to [bt, 128, 2048] fp8.
    # kappa = kt*128 + p; hl = 0 (hi plane) / 1 (lo plane).
    xt0 = nc.dram_tensor("xt0", [BT, 128, 2048], fp8, kind="ExternalInput")
    xt1 = nc.dram_tensor("xt1", [BT, 128, 2048], fp8, kind="ExternalInput")
    # w kpair blocks [R01, R23, -I01, -I23, I01, I23]; each block [p, t, c],
    # kappa = t*128 + p within pair.  Flattened [2, 128, 6144].
    w8 = nc.dram_tensor("w8", [2, 128, 6144], fp8, kind="ExternalInput")
    # out cols: h*1024 + l*128 + o_loc  (h = o-half, l = blade, o = h*128+o_loc)
    out = nc.dram_tensor("out", [BS, OUTW], bf16, kind="ExternalOutput")
    # block index per (output half, kappa-pair j)
    KPIDX = {0: (0, 1, 2, 3),    # re out: [R ; -I]
             1: (4, 5, 0, 1)}    # im out: [I ; R]

    with tile.TileContext(nc) as tc:
        with tc.tile_pool(name="wpool", bufs=1) as wpool, \
             tc.tile_pool(name="xpool", bufs=3) as xpool, \
             tc.tile_pool(name="opool", bufs=3) as opool, \
             tc.tile_pool(name="pspool", bufs=2, space="PSUM") as pspool:
            # PE warmup: ramp the clock gate during the initial DMA wait.
            warm_in = wpool.tile([128, 640], bf16, tag="warm_in")
            nc.vector.memset(warm_in[:], 0.0)
            warm_ps = pspool.tile([128, 512], f32, tag="ps0")
            for _ in range(8):
                nc.tensor.matmul(warm_ps[:], warm_in[:, :128], warm_in[:, 128:640],
                                 start=True, stop=True)

            # Flat w tiles (contiguous DMA APs run at full DMA rate); block j
            # lives at columns [j*1024, (j+1)*1024) viewed [2 t, 512 c].
            wh = wpool.tile([128, 6144], fp8, tag="wh")
            wl = wpool.tile([128, 6144], fp8, tag="wl")
            x0_pre = xpool.tile([128, 2, 8, 128], fp8, tag="x0")
            x1_pre = xpool.tile([128, 2, 8, 128], fp8, tag="x1")
            # Split DMAs: fine-grained deps let the scheduler stream the
            # startup (first matmuls need only x0-hi + wh blocks 0-1).
            nc.sync.dma_start(x0_pre[:, 0], xt0[0][:, 0:1024])      # x0 hi
            for blk in range(0, 6, 2):
                nc.sync.dma_start(wh[:, blk * 1024:(blk + 2) * 1024],
                                  w8[0][:, blk * 1024:(blk + 2) * 1024])
            nc.sync.dma_start(x0_pre[:, 1], xt0[0][:, 1024:2048])   # x0 lo
            for blk in range(0, 6, 2):
                nc.sync.dma_start(wl[:, blk * 1024:(blk + 2) * 1024],
                                  w8[1][:, blk * 1024:(blk + 2) * 1024])
            nc.sync.dma_start(x1_pre[:], xt1[0])

            def chunk_group(xs, ps, h, reim):
                """One accumulation group: psum cols [512h+256reim, +256),
                3 fp8 passes over 4 kappa-pairs."""
                col0 = 512 * h + 256 * reim
                wc0 = 256 * h
                for pi, w_t in enumerate((wh, wh, wl)):
                    hlx = (0, 1, 0)[pi]
                    for j in range(4):
                        blk = KPIDX[reim][j]
                        nc.tensor.matmul(
                            ps[:, col0:col0 + 256],
                            xs[:, hlx, 2 * j:2 * j + 2, :],
                            _apn(w_t[:], blk * 1024 + wc0, [(512, 2), (1, 256)]),
                            start=(pi == 0 and j == 0),
                            stop=(pi == 2 and j == 3),
                            perf_mode=DR)

            def evict(ps, h, tag):
                """psum half h (contiguous cols [512h, 512h+512)) -> SBUF bf16
                [j, r, o] packed (cols j*256 + r*128 + o)."""
                s = opool.tile([128, 512], bf16, tag=tag)
                nc.scalar.copy(
                    _apn(s[:], 0, [(256, 2), (1, 128), (128, 2)]),
                    _apn(ps[:], 512 * h, [(256, 2), (2, 128), (1, 2)]))
                return s

            def bfly(s0, s1, h, stage):
                """8 blades for o-half h from evicted comps, all on DVE
                (packed bf16 2x mode).  stage cols: 1024h + l*128 + o_loc."""
                base = 1024 * h
                o2 = [(1, 128)]
                nc.vector.tensor_add(                      # x0, x7
                    _apn(stage[:], base + 0 * 128, [(896, 2)] + o2),
                    _apn(s0[:], 0, [(256, 2)] + o2),
                    _apn(s1[:], 128, [(256, 2)] + o2))
                nc.vector.tensor_sub(                      # x4, x3
                    _apn(stage[:], base + 4 * 128, [(-128, 2)] + o2),
                    _apn(s0[:], 0, [(256, 2)] + o2),
                    _apn(s1[:], 128, [(256, 2)] + o2))
                nc.vector.tensor_add(                      # x1, x6
                    _apn(stage[:], base + 1 * 128, [(640, 2)] + o2),
                    _apn(s0[:], 128, [(256, 2)] + o2),
                    _apn(s1[:], 0, [(256, 2)] + o2))
                nc.vector.tensor_sub(                      # x5, x2
                    _apn(stage[:], base + 5 * 128, [(-384, 2)] + o2),
                    _apn(s0[:], 128, [(256, 2)] + o2),
                    _apn(s1[:], 0, [(256, 2)] + o2))

            for bt in range(BT):
                last = bt == BT - 1
                if bt == 0:
                    x0_s, x1_s = x0_pre, x1_pre
                else:
                    x0_s = xpool.tile([128, 2, 8, 128], fp8, tag="x0")
                    x1_s = xpool.tile([128, 2, 8, 128], fp8, tag="x1")
                    nc.sync.dma_start(x0_s[:], xt0[bt])
                    nc.sync.dma_start(x1_s[:], xt1[bt])
                ps0 = pspool.tile([128, K], f32, tag="ps0")
                ps1 = pspool.tile([128, K], f32, tag="ps1")
                stage = opool.tile([128, OUTW], bf16, tag="stage")
                row = out[bt * 128:(bt + 1) * 128, :]
                if bt == 0:
                    # w blocks arrive re-first: do re chunks of both halves,
                    # then im (banks alternate, groups stay sequential).
                    for h, reim in ((0, 0), (1, 0), (0, 1), (1, 1)):
                        chunk_group(x0_s, ps0, h, reim)
                    s0a = evict(ps0, 0, "s0a")
                    s0b = evict(ps0, 1, "s0b")
                    for h, reim in ((0, 0), (1, 0), (0, 1), (1, 1)):
                        chunk_group(x1_s, ps1, h, reim)
                    s1a = evict(ps1, 0, "s1a")
                    bfly(s0a, s1a, 0, stage)
                    s1b = evict(ps1, 1, "s1b")
                    bfly(s0b, s1b, 1, stage)
                    nc.sync.dma_start(row, stage[:])
                elif not last:
                    chunk_group(x0_s, ps0, 0, 0)
                    chunk_group(x0_s, ps0, 0, 1)
                    s0a = evict(ps0, 0, "s0a")
                    chunk_group(x0_s, ps0, 1, 0)
                    chunk_group(x0_s, ps0, 1, 1)
                    s0b = evict(ps0, 1, "s0b")
                    chunk_group(x1_s, ps1, 0, 0)
                    chunk_group(x1_s, ps1, 0, 1)
                    s1a = evict(ps1, 0, "s1a")
                    bfly(s0a, s1a, 0, stage)
                    chunk_group(x1_s, ps1, 1, 0)
                    chunk_group(x1_s, ps1, 1, 1)
                    s1b = evict(ps1, 1, "s1b")
                    bfly(s0b, s1b, 1, stage)
                    nc.sync.dma_start(row, stage[:])
                else:
                    # tail: c1 first, drain in column halves so butterfly
                    # and stores overlap the final matmuls.
                    chunk_group(x1_s, ps1, 0, 0)
                    chunk_group(x1_s, ps1, 0, 1)
                    s1a = evict(ps1, 0, "s1a")
                    chunk_group(x1_s, ps1, 1, 0)
                    chunk_group(x1_s, ps1, 1, 1)
                    s1b = evict(ps1, 1, "s1b")
                    chunk_group(x0_s, ps0, 0, 0)
                    chunk_group(x0_s, ps0, 0, 1)
                    s0a = evict(ps0, 0, "s0a")
                    bfly(s0a, s1a, 0, stage)
                    nc.sync.dma_start(row[:, 0:1024], stage[:, 0:1024])
                    chunk_group(x0_s, ps0, 1, 0)
                    chunk_group(x0_s, ps0, 1, 1)
                    s0b = evict(ps0, 1, "s0b")
                    bfly(s0b, s1b, 1, stage)
                    nc.sync.dma_start(row[:, 1024:2048], stage[:, 1024:2048])
    nc.finalize()
    return nc


def _pauli_parts(v):
    """v[..., 8] -> c0, c1 of shape [..., 2(m), 2(reim)]: the c-th column
    (Re, Im) of phi(v) rows m."""
    c0 = np.empty(v.shape[:-1] + (2, 2), dtype=v.dtype)
    c1 = np.empty_like(c0)
    v0, v1, v2, v3, v4, v5, v6, v7 = (v[..., a] for a in range(8))
    c0[..., 0, 0] = v0 + v4   # Re A
    c0[..., 0, 1] = v3 + v7   # Im A
    c0[..., 1, 0] = v1 + v5   # Re C
    c0[..., 1, 1] = v6 + v2   # Im C
    c1[..., 0, 0] = v1 - v5   # Re B
    c1[..., 0, 1] = v6 - v2   # Im B
    c1[..., 1, 0] = v0 - v4   # Re D
    c1[..., 1, 1] = v7 - v3   # Im D
    return c0, c1


def _hi_lo(v):
    """f32 array -> (hi, lo) e4m3 planes with hi + lo ~= v."""
    hi = v.astype(E4)
    lo = (v - hi.astype(np.float32)).astype(E4)
    return hi, lo


def _prep_w(weight):
    """weight [COUT, CIN, 8] -> [2, 128, 6144] fp8 kpair blocks
    [R01, R23, -I01, -I23, I01, I23], each [128 p, 2 t, 512 c]."""
    w = weight.astype(np.float32)
    cw0, cw1 = _pauli_parts(w)
    R = np.empty((CIN, 2, COUT, 2), np.float32)   # [(i,m),(o,r)]
    I = np.empty_like(R)
    for m, cm in ((0, cw0), (1, cw1)):
        for r in range(2):
            R[:, m, :, r] = 0.5 * cm[:, :, r, 0].T
            I[:, m, :, r] = 0.5 * cm[:, :, r, 1].T
    R = R.reshape(HK, HK) * SW
    I = I.reshape(HK, HK) * SW
    out = np.empty((2, 128, 6144), dtype=E4)
    for hl in (0, 1):
        blocks = []
        for M in (R, -I, I):
            h, l = _hi_lo(M)
            P = (h if hl == 0 else l).astype(np.float32)
            for j in (0, 1):
                blk = P[256 * j:256 * j + 256].reshape(2, 128, HK)
                blocks.append(blk.transpose(1, 0, 2))   # [128, 2, 512]
        arr = np.stack(blocks, axis=1)    # [128, 6, 2, 512]
        out[hl] = arr.reshape(128, 6144).astype(E4)
    return out


def _prep_x(x):
    """x [B, CIN, 8] -> two per-core arrays [N_CORES, BT, 128, 2048] fp8
    (c = 0, 1) in [bt, p, hl, kt, b] layout; kappa = kt*128 + p."""
    xf = x.astype(np.float32)
    c0, c1 = _pauli_parts(xf)          # [B, CIN, m, reim]
    outs = []
    for arr in (c0, c1):
        kb = arr.transpose(3, 1, 2, 0).reshape(K, B) * SX   # [K, B]
        a = kb.reshape(KT, 128, N_CORES, BT, 128)   # [kt, p, core, bt, b]
        a = np.ascontiguousarray(a.transpose(2, 3, 1, 0, 4))  # [core,bt,p,kt,b]
        hi, lo = _hi_lo(a)
        packed = np.stack([hi, lo], axis=3)  # [core, bt, p, hl, kt, b]
        outs.append(np.ascontiguousarray(
            packed.reshape(N_CORES, BT, 128, 2048)))
    return outs


def kernel(x, weight, bias, cayley):
    assert x.shape == (B, CIN, NB) and weight.shape == (COUT, CIN, NB)
    if "nc" not in _cached:
        _cached["nc"] = _build_nc()
    nc = _cached["nc"]

    xt0, xt1 = _prep_x(np.asarray(x))
    w8 = _prep_w(np.asarray(weight))
    in_maps = [{"xt0": xt0[c], "xt1": xt1[c], "w8": w8} for c in range(N_CORES)]
    res = run_bass_kernel_spmd(nc, in_maps, core_ids=list(range(N_CORES)))
    out = np.concatenate(
        [res.results[c]["out"].astype(np.float32) for c in range(N_CORES)],
        axis=0)
    # device cols: (h, l, o_loc) -> out[b, o, l] with o = h*128 + o_loc
    out = out.reshape(B, 2, NB, 128).transpose(0, 1, 3, 2).reshape(B, COUT, NB)
    out = out * (1.0 / (SX * SW))
    out = out + np.asarray(bias, np.float32)[None]
    return out.astype(np.float32)


# revision 9
# speedup vs baseline: 1.5486x; 1.2327x over previous
"""CliffordLinear (Cl(3,0)) Trainium2 kernel — fp8 DoubleRow + Gauss edition.

Math: Cl(3,0) ~= 2x2 complex matrices via Pauli rep phi.  The reference's
per-channel Clifford contraction maps to OutM[b,o] = sum_i phi(W[o,i]) @
phi(X[b,i]).  Per output column c in {0,1}, with A = phi(W) as 512x512
[(i,m) x (o,r)] planes Ar/Ai and x column halves xr/xi, the 3-multiply
(Karatsuba/Gauss) form shares one product between Re and Im:

    k1 = Ar @ (xr + xi)      k2' = -(Ar+Ai) @ xi      k3 = (Ai-Ar) @ xr
    Re = k1 + k2'            Im = k1 + k3

k2' and k3 accumulate in PSUM; k1 is evicted once (ScalarE) and added to
both via one broadcast (stride-0) DVE op per column half -- 3/4 the PE
work of the plain 4-multiply form at the cost of 1.5x x traffic.

Precision/performance: all matmul operands are fp8e4 (e4m3) hi+lo pairs
prepared on the host (v ~= hi + lo, ~13 mantissa bits); each product runs
three fp8 DoubleRow passes (hi*hi + lo*hi + hi*lo), contracting 256 rows
per matmul at 0.5 cycles/row.  Inputs stream as fp8 pairs, outputs as
bf16.  The inverse-Pauli butterfly runs on packed bf16 APs (DVE 2x mode +
Pool), and the last b-tile drains c1-before-c0 in column halves so the
tail overlaps the final matmuls.

Sharding: data-parallel over batch (1024 rows/core); weights replicated.
Per-core HBM traffic: 6.3 MB x + 1.5 MB w in, 4.2 MB out.
"""

import sys

sys.path.insert(0, "/opt/trn_rl_repo")

import numpy as np
import ml_dtypes

import concourse.bass as bass  # noqa: F401  (registers lowerings)
import concourse.mybir as mybir
import concourse.tile as tile
from concourse import bacc
from concourse.bass_utils import run_bass_kernel_spmd

N_CORES = 8
B, CIN, COUT, NB = 8192, 256, 256, 8
BS = B // N_CORES          # 1024 batch rows per core
K = CIN * 4                # 1024 contraction rows (re|im halves)
HK = K // 2                # 512 rows per half (one Gauss operand)
OUTW = COUT * NB           # 2048 output width
BT = BS // 128             # 8 b-tiles
E4 = ml_dtypes.float8_e4m3
SX = 16.0                  # x scale before e4m3 quantization
SW = 1024.0                # w scale before e4m3 quantization

_cached = {}


def _apn(base, off, dims):
    """AP with explicit free dims [(step, n), ...] at column offset off."""
    a = base.copy()
    part = a.ap.to_list()[0]
    v = a.ap
    v.clear()
    v.extend([tuple(part)] + [tuple(d) for d in dims])
    a.offset = a.offset + off
    return a


def _build_nc():
    f32 = mybir.dt.float32
    bf16 = mybir.dt.bfloat16
    fp8 = mybir.dt.float8e4
    DR = mybir.MatmulPerfMode.DoubleRow
    nc = bacc.Bacc("TRN2", target_bir_lowering=False, debug=False,
                   num_devices=N_CORES)
    # x'[c]: [bt, p, hl, op, kt, b] flat [bt, 128, 3072] fp8;
    # op 0/1/2 = xs (xr+xi) / xi / xr; row kappa = kt*128 + p (4 kt per op).
    xt0 = nc.dram_tensor("xt0", [BT, 128, 3072], fp8, kind="ExternalInput")
    xt1 = nc.dram_tensor("xt1", [BT, 128, 3072], fp8, kind="ExternalInput")
    # w kpair blocks [P1_01, P1_23, P2_01, P2_23, P3_01, P3_23] for planes
    # P1 = Ar, P2 = -(Ar+Ai), P3 = Ai-Ar; each block [p, t, c].
    w8 = nc.dram_tensor("w8", [2, 128, 6144], fp8, kind="ExternalInput")
    # out cols: h*1024 + l*128 + o_loc  (h = o-half, l = blade)
    out = nc.dram_tensor("out", [BS, OUTW], bf16, kind="ExternalOutput")

    with tile.TileContext(nc) as tc:
        with tc.tile_pool(name="wpool", bufs=1) as wpool, \
             tc.tile_pool(name="xpool", bufs=3) as xpool, \
             tc.tile_pool(name="opool", bufs=3) as opool, \
             tc.tile_pool(name="pspool", bufs=2, space="PSUM") as pspool:
            # PE warmup: ramp the clock gate during the initial DMA wait.
            warm_in = wpool.tile([128, 640], bf16, tag="warm_in")
            nc.vector.memset(warm_in[:], 0.0)
            warm_ps = pspool.tile([128, 512], f32, tag="pk0")
            for _ in range(8):
                nc.tensor.matmul(warm_ps[:], warm_in[:, :128], warm_in[:, 128:640],
                                 start=True, stop=True)

            wh = wpool.tile([128, 6144], fp8, tag="wh")
            wl = wpool.tile([128, 6144], fp8, tag="wl")
            x0_pre = xpool.tile([128, 2, 3, 4, 128], fp8, tag="x0")
            x1_pre = xpool.tile([128, 2, 3, 4, 128], fp8, tag="x1")
            nc.sync.dma_start(x0_pre[:, 0], xt0[0][:, 0:1536])      # x0 hi
            for blk in range(0, 6, 2):
                nc.sync.dma_start(wh[:, blk * 1024:(blk + 2) * 1024],
                                  w8[0][:, blk * 1024:(blk + 2) * 1024])
            nc.sync.dma_start(x0_pre[:, 1], xt0[0][:, 1536:3072])   # x0 lo
            for blk in range(0, 6, 2):
                nc.sync.dma_start(wl[:, blk * 1024:(blk + 2) * 1024],
                                  w8[1][:, blk * 1024:(blk + 2) * 1024])
            nc.sync.dma_start(x1_pre[:], xt1[0])

            def half_groups(xs, pk, h):
                """k1/k2'/k3 accumulation groups for one (c, o-half):
                pk cols [g*256, g*256+256), w cols [256h, 256h+256)."""
                wc0 = 256 * h
                for g in range(3):
                    col0 = 256 * g
                    for pi, w_t in enumerate((wh, wh, wl)):
                        hlx = (0, 1, 0)[pi]
                        for j in range(2):
                            nc.tensor.matmul(
                                pk[:, col0:col0 + 256],
                                xs[:, hlx, g, 2 * j:2 * j + 2, :],
                                _apn(w_t[:], (2 * g + j) * 1024 + wc0,
                                     [(512, 2), (1, 256)]),
                                start=(pi == 0 and j == 0),
                                stop=(pi == 2 and j == 1),
                                perf_mode=DR)

            def evict_k1(pk, tag):
                """k1 (pk cols [0,256)) -> SBUF bf16 [r, o] (col r*128+o)."""
                s = opool.tile([128, 256], bf16, tag=tag)
                nc.scalar.copy(
                    _apn(s[:], 0, [(1, 128), (128, 2)]),
                    _apn(pk[:], 0, [(2, 128), (1, 2)]))
                return s

            def recomb(pk, sk1, tag):
                """comps [j, r, o] bf16 (col j*256 + r*128 + o):
                Re = k2' + k1, Im = k3 + k1 in one broadcast dual op."""
                comps = opool.tile([128, 512], bf16, tag=tag)
                nc.vector.tensor_add(
                    _apn(comps[:], 0, [(256, 2), (1, 128), (128, 2)]),
                    _apn(pk[:], 256, [(256, 2), (2, 128), (1, 2)]),
                    _apn(sk1[:], 0, [(0, 2), (1, 128), (128, 2)]))
                return comps

            def bfly(c0, c1, h, stage, tail=False):
                """8 blades for o-half h; stage cols 1024h + l*128 + o_loc.
                DVE+Pool split mid-run, all-DVE on the tail."""
                base = 1024 * h
                o2 = [(1, 128)]
                sub2 = nc.vector.tensor_sub if tail else nc.gpsimd.tensor_sub
                nc.vector.tensor_add(                      # x0, x7
                    _apn(stage[:], base + 0 * 128, [(896, 2)] + o2),
                    _apn(c0[:], 0, [(256, 2)] + o2),
                    _apn(c1[:], 128, [(256, 2)] + o2))
                sub2(                                      # x4, x3
                    _apn(stage[:], base + 4 * 128, [(-128, 2)] + o2),
                    _apn(c0[:], 0, [(256, 2)] + o2),
                    _apn(c1[:], 128, [(256, 2)] + o2))
                nc.vector.tensor_add(                      # x1, x6
                    _apn(stage[:], base + 1 * 128, [(640, 2)] + o2),
                    _apn(c0[:], 128, [(256, 2)] + o2),
                    _apn(c1[:], 0, [(256, 2)] + o2))
                sub2(                                      # x5, x2
                    _apn(stage[:], base + 5 * 128, [(-384, 2)] + o2),
                    _apn(c0[:], 128, [(256, 2)] + o2),
                    _apn(c1[:], 0, [(256, 2)] + o2))

            def process_half(xs, h, ctag):
                """matmuls + k1 eviction + recombination for one (c, half)."""
                pk = pspool.tile([128, 768], f32, tag=f"pk{ctag}")
                half_groups(xs, pk, h)
                sk1 = evict_k1(pk, f"sk{ctag}{h}")
                return recomb(pk, sk1, f"c{ctag}{h}")

            for bt in range(BT):
                last = bt == BT - 1
                if bt == 0:
                    x0_s, x1_s = x0_pre, x1_pre
                else:
                    x0_s = xpool.tile([128, 2, 3, 4, 128], fp8, tag="x0")
                    x1_s = xpool.tile([128, 2, 3, 4, 128], fp8, tag="x1")
                    nc.sync.dma_start(x0_s[:], xt0[bt])
                    nc.sync.dma_start(x1_s[:], xt1[bt])
                stage = opool.tile([128, OUTW], bf16, tag="stage")
                row = out[bt * 128:(bt + 1) * 128, :]
                if not last:
                    c0a = process_half(x0_s, 0, 0)
                    c0b = process_half(x0_s, 1, 0)
                    c1a = process_half(x1_s, 0, 1)
                    bfly(c0a, c1a, 0, stage)
                    c1b = process_half(x1_s, 1, 1)
                    bfly(c0b, c1b, 1, stage)
                    nc.sync.dma_start(row, stage[:])
                else:
                    # tail: c1 first, drain in column halves.
                    c1a = process_half(x1_s, 0, 1)
                    c1b = process_half(x1_s, 1, 1)
                    c0a = process_half(x0_s, 0, 0)
                    bfly(c0a, c1a, 0, stage, tail=True)
                    nc.sync.dma_start(row[:, 0:1024], stage[:, 0:1024])
                    c0b = process_half(x0_s, 1, 0)
                    bfly(c0b, c1b, 1, stage, tail=True)
                    nc.sync.dma_start(row[:, 1024:2048], stage[:, 1024:2048])
    nc.finalize()
    return nc


def _pauli_parts(v):
    """v[..., 8] -> c0, c1 of shape [..., 2(m), 2(reim)]: the c-th column
    (Re, Im) of phi(v) rows m."""
    c0 = np.empty(v.shape[:-1] + (2, 2), dtype=v.dtype)
    c1 = np.empty_like(c0)
    v0, v1, v2, v3, v4, v5, v6, v7 = (v[..., a] for a in range(8))
    c0[..., 0, 0] = v0 + v4   # Re A
    c0[..., 0, 1] = v3 + v7   # Im A
    c0[..., 1, 0] = v1 + v5   # Re C
    c0[..., 1, 1] = v6 + v2   # Im C
    c1[..., 0, 0] = v1 - v5   # Re B
    c1[..., 0, 1] = v6 - v2   # Im B
    c1[..., 1, 0] = v0 - v4   # Re D
    c1[..., 1, 1] = v7 - v3   # Im D
    return c0, c1


def _hi_lo(v):
    """f32 array -> (hi, lo) e4m3 planes with hi + lo ~= v."""
    hi = v.astype(E4)
    lo = (v - hi.astype(np.float32)).astype(E4)
    return hi, lo


def _prep_w(weight):
    """weight [COUT, CIN, 8] -> [2, 128, 6144] fp8 kpair blocks of the
    Gauss planes [Ar, -(Ar+Ai), Ai-Ar], each block [128 p, 2 t, 512 c]."""
    w = weight.astype(np.float32)
    cw0, cw1 = _pauli_parts(w)
    R = np.empty((CIN, 2, COUT, 2), np.float32)   # [(i,m),(o,r)]
    I = np.empty_like(R)
    for m, cm in ((0, cw0), (1, cw1)):
        for r in range(2):
            R[:, m, :, r] = 0.5 * cm[:, :, r, 0].T
            I[:, m, :, r] = 0.5 * cm[:, :, r, 1].T
    R = R.reshape(HK, HK) * SW
    I = I.reshape(HK, HK) * SW
    out = np.empty((2, 128, 6144), dtype=E4)
    for hl in (0, 1):
        blocks = []
        for M in (R, -(R + I), I - R):
            h, l = _hi_lo(M)
            P = (h if hl == 0 else l).astype(np.float32)
            for j in (0, 1):
                blk = P[256 * j:256 * j + 256].reshape(2, 128, HK)
                blocks.append(blk.transpose(1, 0, 2))   # [128, 2, 512]
        arr = np.stack(blocks, axis=1)    # [128, 6, 2, 512]
        out[hl] = arr.reshape(128, 6144).astype(E4)
    return out


def _prep_x(x):
    """x [B, CIN, 8] -> two per-core arrays [N_CORES, BT, 128, 3072] fp8
    (c = 0, 1): ops [xs, xi, xr] in [bt, p, hl, op, kt, b] layout."""
    xf = x.astype(np.float32)
    c0, c1 = _pauli_parts(xf)          # [B, CIN, m, reim]
    outs = []
    for arr in (c0, c1):
        kb = arr.transpose(3, 1, 2, 0).reshape(K, B) * SX   # [K, B]
        xr, xi = kb[0:HK], kb[HK:K]
        ops = np.stack([xr + xi, xi, xr], axis=0)   # [op, 512, B]
        a = ops.reshape(3, 4, 128, N_CORES, BT, 128)  # [op, kt, p, core, bt, b]
        a = np.ascontiguousarray(a.transpose(3, 4, 2, 0, 1, 5))
        hi, lo = _hi_lo(a)                       # [core, bt, p, op, kt, b]
        packed = np.stack([hi, lo], axis=3)      # [core, bt, p, hl, op, kt, b]
        outs.append(np.ascontiguousarray(
            packed.reshape(N_CORES, BT, 128, 3072)))
    return outs


def kernel(x, weight, bias, cayley):
    assert x.shape == (B, CIN, NB) and weight.shape == (COUT, CIN, NB)
    if "nc" not in _cached:
        _cached["nc"] = _build_nc()
    nc = _cached["nc"]

    xt0, xt1 = _prep_x(np.asarray(x))
    w8 = _prep_w(np.asarray(weight))
    in_maps = [{"xt0": xt0[c], "xt1": xt1[c], "w8": w8} for c in range(N_CORES)]
    res = run_bass_kernel_spmd(nc, in_maps, core_ids=list(range(N_CORES)))
    out = np.concatenate(
        [res.results[c]["out"].astype(np.float32) for c in range(N_CORES)],
        axis=0)
    # device cols: (h, l, o_loc) -> out[b, o, l] with o = h*128 + o_loc
    out = out.reshape(B, 2, NB, 128).transpose(0, 1, 3, 2).reshape(B, COUT, NB)
    out = out * (1.0 / (SX * SW))
    out = out + np.asarray(bias, np.float32)[None]
    return out.astype(np.float32)
